# revision 98
# baseline (speedup 1.0000x reference)
"""GQA attention with LoRA-Q, tensor-parallel over 8 TRN2 cores.

Sharding (per core i of 8):
  - Q heads 4i..4i+3 (256 q-dims) and KV head i (GQA: repeat_interleave maps
    q heads [4i,4i+4) exactly onto kv head i).
  - Wq (with LoRA folded: Wq_eff = Wq + lora_B @ lora_A), Wk, Wv row-sharded;
    Wo column-sharded on its input (head) dim.
  - Each core computes a full-width PARTIAL output y_part = O_loc @ Wo_loc^T
    [T, D]; per-T-chunk ReduceScatter(add) over rows gives core i rows
    512c+64i..+64 — 8x less collective traffic than AllGathering O, and the
    first three collectives overlap attention compute.

Single fused pipeline over T-chunks of 512: QKV-proj(c) -> RoPE(c) ->
attention(c) -> Wo-partial(c-1), with the causal mask applied as a -240
bias added into the score PSUM by the tensor engine (exp then yields ~0),
and score/PV/exp work triangularly trimmed on diagonal blocks.

All matmuls in bf16 with fp32 PSUM accumulation; softmax without max
subtraction (scores are bounded: |S/8| <= ~7), denominator fused into the
PV matmul via an appended ones-column on V.
"""

import numpy as np
import ml_dtypes

import concourse.bass as bass
import concourse.mybir as mybir
import concourse.tile as tile
from concourse import bacc
from concourse.bass_utils import run_bass_kernel_spmd
from concourse.masks import make_identity

BF16 = mybir.dt.bfloat16
F32 = mybir.dt.float32

N_CORES = 8
T = 2048
D = 2048
HD = 64          # head dim
NH = 32          # total q heads
NKV = 8          # total kv heads
NH_LOC = NH // N_CORES       # 4 q heads per core
QW = NH_LOC * HD             # 256 q dims per core
P = 128
KT = D // P                  # 16 contraction tiles
CH = 512         # T-chunk (psum free dim)
NCH = T // CH                # 4 chunks
NJ = T // P                  # 16 k-blocks
SCALE = 1.0 / 8.0            # 1/sqrt(64)
TR = T // N_CORES            # 256 output rows per core after ReduceScatter
NEG = -240.0                 # additive causal-mask bias (exp(-30) ~ 0)


def build_bass():
    nc = bacc.Bacc(None, num_devices=N_CORES)

    # I/O
    xT_d = nc.dram_tensor("xT", [D, T], BF16, kind="ExternalInput")
    w_d = nc.dram_tensor("w_all", [D, QW + 2 * HD], BF16, kind="ExternalInput")
    woT_d = nc.dram_tensor("woT", [QW, D], BF16, kind="ExternalInput")
    # extras: [R128 perm | mask bias | cos | sin] along free dim
    ex_d = nc.dram_tensor("extras", [P, 2 * P + 2 * T], BF16, kind="ExternalInput")
    y_d = nc.dram_tensor("y", [TR, D], BF16, kind="ExternalOutput")

    with tile.TileContext(nc, num_cores=N_CORES) as tc:
        _body(nc, tc, xT_d, w_d, woT_d, ex_d, y_d)
    nc.compile()
    return nc


def _body(nc, tc, xT_d, w_d, woT_d, ex_d, y_d):
    import contextlib

    ctx = contextlib.ExitStack()
    with ctx:
        consts = ctx.enter_context(tc.tile_pool(name="consts", bufs=1))
        big = ctx.enter_context(tc.tile_pool(name="big", bufs=1))
        work = ctx.enter_context(tc.tile_pool(name="work", bufs=1))
        rope_p = ctx.enter_context(tc.tile_pool(name="rope_p", bufs=2))
        yp_p = ctx.enter_context(tc.tile_pool(name="yp_p", bufs=3))
        pt_p = ctx.enter_context(tc.tile_pool(name="pt_p", bufs=3))
        rcp_p = ctx.enter_context(tc.tile_pool(name="rcp_p", bufs=3))
        psum_st = ctx.enter_context(tc.tile_pool(name="psum_st", bufs=2, space="PSUM"))
        psum_o = ctx.enter_context(tc.tile_pool(name="psum_o", bufs=2, space="PSUM"))
        dram = ctx.enter_context(tc.tile_pool(name="dram", bufs=1, space="DRAM"))

        # ---- loads: few large DMAs (per-DMA HWDGE setup ~625ns dominates
        # small transfers; bus runs at full speed on >=512B descriptors)
        w_sb = consts.tile([P, KT, QW + 2 * HD], BF16)
        w_r = w_d.rearrange("(kt p) m -> p kt m", p=P)
        xT_sb = big.tile([P, KT, T], BF16, tag="xT")
        xT_r = xT_d.rearrange("(kt p) t -> p kt t", p=P)
        ex_sb = consts.tile([P, 2 * P + 2 * T], BF16)
        nc.sync.dma_start(w_sb[:, 0:4, :], w_r[:, 0:4, :])
        nc.sync.dma_start(xT_sb[:, 0:4, 0:CH], xT_r[:, 0:4, 0:CH])
        nc.sync.dma_start(w_sb[:, 4:16, :], w_r[:, 4:16, :])
        nc.sync.dma_start(xT_sb[:, 4:8, 0:CH], xT_r[:, 4:8, 0:CH])
        nc.sync.dma_start(xT_sb[:, 8:12, 0:CH], xT_r[:, 8:12, 0:CH])
        nc.sync.dma_start(xT_sb[:, 12:16, 0:CH], xT_r[:, 12:16, 0:CH])
        nc.sync.dma_start(ex_sb, ex_d[:])
        for c in range(1, NCH):
            sl = slice(c * CH, (c + 1) * CH)
            nc.sync.dma_start(xT_sb[:, :, sl], xT_r[:, :, sl])
        R128 = ex_sb[:, 0:P]
        mbias = ex_sb[:, P : 2 * P]
        cos2 = ex_sb[:, 2 * P : 2 * P + T]
        sin2 = ex_sb[:, 2 * P + T : 2 * P + 2 * T]
        woT_sb = consts.tile([P, 2, D], BF16)
        woT_r = woT_d.rearrange("(kh p) d -> p kh d", p=P)
        nc.sync.dma_start(woT_sb, woT_r)

        ident64 = consts.tile([HD, HD], BF16)
        make_identity(nc, ident64)
        ident128 = consts.tile([P, P], BF16)
        make_identity(nc, ident128)
        ones64 = consts.tile([1, HD], BF16)
        nc.vector.memset(ones64, 1.0)

        # v with ones column appended: [tk(P), j, HD+1]
        v_aug = work.tile([P, NJ, HD + 1], BF16)
        nc.vector.memset(v_aug[:, :, HD : HD + 1], 1.0)

        projT = work.tile([P, 3, T], BF16)     # m=0: heads 0,1; m=1: heads 2,3
        qT128 = work.tile([P, 2, T], BF16)     # RoPE'd q, same packing
        # kT duplicated into both partition halves so ST lhsT base can match
        # the q operand's base for odd heads
        kT_sb = work.tile([P, T], BF16)
        OT128 = work.tile([P, 2, T], BF16)     # normalized O^T, row kh*128+p
        ypart_dram = dram.tile([T, D], BF16)
        ypart_r = ypart_dram.rearrange("(mt p) d -> p mt d", p=P)
        y_rs = dram.tile([TR, D], BF16)

        def cch(c):
            return slice(c * CH, (c + 1) * CH)

        def proj_fillers(c):
            # chunk-c projection split into 4-kt filler items (one psum
            # group per m stays open across items; copy rides the last)
            items = []

            def piece(m, ktg):
                def emit():
                    if ktg == 0:
                        box[m] = psum_o.tile([P, CH], F32, tag="mm", name=f"pj{m}")
                    ps = box[m]
                    for kt in range(4 * ktg, 4 * ktg + 4):
                        nc.tensor.matmul(
                            ps,
                            lhsT=w_sb[:, kt, m * P : (m + 1) * P],
                            rhs=xT_sb[:, kt, cch(c)],
                            start=(kt == 0),
                            stop=(kt == KT - 1),
                        )
                    if ktg == 3:
                        nc.vector.tensor_copy(projT[:, m, cch(c)], ps)

                return emit

            box = {}
            for m in (0, 1, 2):
                for ktg in range(4):
                    items.append(piece(m, ktg))
            return items

        def proj_rope_chunk(c, skip_proj=False):
            # K/V block (m=2) first so k-RoPE overlaps the q projections;
            # each q pair's RoPE is emitted as soon as its projT slice lands.
            # All elementwise RoPE work on DVE: the Pool queue must stay clear
            # for collectives (a waiting collective blocks later Pool ops).
            if not skip_proj:
                for m in (0, 1, 2):
                    ps = psum_o.tile([P, CH], F32, tag="mm")
                    for kt in range(KT):
                        nc.tensor.matmul(
                            ps,
                            lhsT=w_sb[:, kt, m * P : (m + 1) * P],
                            rhs=xT_sb[:, kt, cch(c)],
                            start=(kt == 0),
                            stop=(kt == KT - 1),
                        )
                    nc.scalar.copy(projT[:, m, cch(c)], ps)
            for s in range(2):
                qs = psum_o.tile([P, CH], F32, tag="ot")
                nc.tensor.matmul(
                    qs, lhsT=R128, rhs=projT[:, s, cch(c)], start=True, stop=True
                )
                t1 = rope_p.tile([P, CH], BF16, tag="t1")
                nc.vector.tensor_mul(t1, projT[:, s, cch(c)], cos2[:, cch(c)])
                t2 = rope_p.tile([P, CH], BF16, tag="t2")
                nc.vector.tensor_mul(t2, qs, sin2[:, cch(c)])
                nc.vector.tensor_add(qT128[:, s, cch(c)], t1, t2)
            ks = psum_o.tile([P, CH], F32, tag="ot")
            nc.tensor.matmul(
                ks[0:HD, :],
                lhsT=R128[0:HD, 0:HD],
                rhs=projT[0:HD, 2, cch(c)],
                start=True,
                stop=True,
            )
            k1 = rope_p.tile([HD, CH], BF16, tag="k1")
            nc.vector.tensor_mul(k1, projT[0:HD, 2, cch(c)], cos2[0:HD, cch(c)])
            k2 = rope_p.tile([HD, CH], BF16, tag="k2")
            nc.vector.tensor_mul(k2, ks[0:HD, :], sin2[0:HD, cch(c)])
            nc.vector.tensor_add(kT_sb[0:HD, cch(c)], k1, k2)
            nc.vector.tensor_copy(kT_sb[HD:P, cch(c)], kT_sb[0:HD, cch(c)])

        def vt_fillers(c):
            # v transpose for chunk c's k-blocks; only the diagonal PVs (the
            # tail of each head's unit list) need these
            def item(j):
                def emit():
                    tp = psum_o.tile([P, CH], BF16, tag="mm")
                    nc.tensor.transpose(
                        tp[:, 0:HD],
                        projT[HD:P, 2, j * P : (j + 1) * P],
                        ident128[HD:P, HD:P],
                    )
                    nc.vector.tensor_copy(v_aug[:, j, 0:HD], tp[:, 0:HD])

                return emit

            return [item(j) for j in range(4 * c, 4 * c + 4)]

        pending_norm = []
        proj1 = proj_fillers(1)

        def flush_norm(n):
            # softmax normalization: recip of denominator row, broadcast via
            # PE, multiply unnormalized O rows into OT128 (on [lo:hi) cols)
            for h, c, ot, lo, hi in pending_norm[:n]:
                w = hi - lo
                rrow = rcp_p.tile([1, CH], BF16, tag="rrow")
                with nc.allow_low_precision("softmax denom in bf16 is fine"):
                    nc.vector.reciprocal(rrow[:, 0:w], ot[HD : HD + 1, lo:hi])
                bc = psum_o.tile([P, CH], F32, tag="mm")
                nc.tensor.matmul(
                    bc[0:HD, 0:w],
                    lhsT=ones64,
                    rhs=rrow[:, 0:w],
                    start=True,
                    stop=True,
                )
                bcs = rcp_p.tile([HD, CH], BF16, tag="bcs")
                nc.vector.tensor_copy(bcs[:, 0:w], bc[0:HD, 0:w])
                hp = (h % 2) * HD
                nc.vector.tensor_mul(
                    OT128[hp : hp + HD, h // 2, c * CH + lo : c * CH + hi],
                    ot[0:HD, lo:hi],
                    bcs[:, 0:w],
                )
            del pending_norm[:n]

        def attn_head(h, c, filler):
            # units: [(j_or_r list, kind)] — off-diagonal pairs then the two
            # diagonal pairs; per-unit: ST (+bias on diag) -> exp -> PV.
            ot = psum_o.tile([P, CH], F32, tag="ot")
            units = []
            off = list(range(0, 4 * c))
            for g in range(0, len(off), 2):
                units.append(("off", off[g : g + 2]))
            units.append(("diag", [0, 1]))
            units.append(("diag", [2, 3]))

            hb = (h % 2) * HD   # partition base of this head's q rows

            def do_st(kind, js):
                st = psum_st.tile([P, 2, CH], F32, tag="st")
                if kind == "off":
                    for idx, j in enumerate(js):
                        nc.tensor.matmul(
                            st[:, idx, :],
                            lhsT=kT_sb[hb : hb + HD, j * P : (j + 1) * P],
                            rhs=qT128[hb : hb + HD, h // 2, cch(c)],
                            start=True,
                            stop=True,
                        )
                else:
                    for idx, r in enumerate(js):
                        j = 4 * c + r
                        q0 = P * r
                        nc.tensor.matmul(
                            st[:, idx, q0:CH],
                            lhsT=kT_sb[hb : hb + HD, j * P : (j + 1) * P],
                            rhs=qT128[
                                hb : hb + HD,
                                h // 2,
                                c * CH + q0 : (c + 1) * CH,
                            ],
                            start=True,
                            stop=False,
                            skip_group_check=True,
                        )
                        nc.tensor.matmul(
                            st[:, idx, q0 : q0 + P],
                            lhsT=ident128,
                            rhs=mbias,
                            start=False,
                            stop=True,
                            skip_group_check=True,
                        )
                return st

            def do_rest(kind, js, st):
                pt = pt_p.tile([P, 2, CH], BF16, tag="pt")
                if kind == "off":
                    nc.scalar.activation(
                        pt, st, mybir.ActivationFunctionType.Exp, scale=SCALE
                    )
                    for idx, j in enumerate(js):
                        nc.tensor.matmul(
                            ot[0 : HD + 1, :],
                            lhsT=v_aug[:, j, :],
                            rhs=pt[:, idx, :],
                            start=(j == 0),
                            stop=False,
                            skip_group_check=True,
                        )
                else:
                    # one exp over both blocks at the union of their valid
                    # column ranges; the stale sub-range of the second block
                    # is never read by its PV matmul
                    q0u = P * js[0]
                    nc.scalar.activation(
                        pt[:, :, q0u:CH],
                        st[:, :, q0u:CH],
                        mybir.ActivationFunctionType.Exp,
                        scale=SCALE,
                    )
                    for idx, r in enumerate(js):
                        j = 4 * c + r
                        q0 = P * r
                        nc.tensor.matmul(
                            ot[0 : HD + 1, q0:CH],
                            lhsT=v_aug[:, j, :],
                            rhs=pt[:, idx, q0:CH],
                            start=(c == 0 and r == 0),
                            stop=(r == 3),
                            skip_group_check=True,
                        )

            st_cur = do_st(*units[0])
            for u in range(len(units)):
                st_next = do_st(*units[u + 1]) if u + 1 < len(units) else None
                if filler:
                    filler.pop(0)()  # independent PE work to cover exp(u)
                do_rest(units[u][0], units[u][1], st_cur)
                if u + 1 == len(units) and filler and (c < 3 or h < 2):
                    filler.pop(0)()  # cover the head-boundary hop
                st_cur = st_next
            if c == NCH - 1 and h == NH_LOC - 1:
                # split the last head's norm so the first half of the final
                # Wo-partial launches before the whole chunk is normalized
                pending_norm.append((h, c, ot, 0, 2 * P))
                pending_norm.append((h, c, ot, 2 * P, CH))
            else:
                pending_norm.append((h, c, ot, 0, CH))

        def ypart_fillers(c, tail=False):
            # y_part rows of chunk c: [512, D] = O_loc^T-slice^T @ Wo_loc^T.
            # Returned as a list of small closures so the PE work can be
            # sprinkled between attention units (fills exp-wait bubbles).
            ypb = yp_p.tile([P, 4, D], BF16, tag="yp")

            def group(i, mt, dc):
                # two filler items per (mt, dc): half-width matmul pairs into
                # one psum tile, copy attached to the second half
                box = {}

                def emit_a():
                    # in the tail block the attention psum banks are idle:
                    # alternate tags for a 4-deep rotation
                    if tail and (mt + dc) % 2 == 1:
                        ps = psum_st.tile([P, CH], F32, tag="st", name=f"yp{mt}_{dc}")
                    else:
                        ps = psum_o.tile([P, CH], F32, tag="mm", name=f"yp{mt}_{dc}")
                    box["ps"] = ps
                    for kh in range(2):
                        nc.tensor.matmul(
                            box["ps"][:, 0 : CH // 2],
                            lhsT=OT128[:, kh, mt * P : (mt + 1) * P],
                            rhs=woT_sb[:, kh, dc * CH : dc * CH + CH // 2],
                            start=(kh == 0),
                            stop=(kh == 1),
                            skip_group_check=True,
                        )

                def emit_b():
                    ps = box["ps"]
                    for kh in range(2):
                        nc.tensor.matmul(
                            ps[:, CH // 2 : CH],
                            lhsT=OT128[:, kh, mt * P : (mt + 1) * P],
                            rhs=woT_sb[:, kh, dc * CH + CH // 2 : (dc + 1) * CH],
                            start=(kh == 0),
                            stop=(kh == 1),
                            skip_group_check=True,
                        )
                    if dc % 2 == 0:
                        nc.vector.tensor_copy(ypb[:, i, cch(dc)], ps)
                    else:
                        nc.scalar.copy(ypb[:, i, cch(dc)], ps)

                return [emit_a, emit_b]

            def dma_half(half):
                pieces = [(0, 2), (2, 3), (3, 4)] if tail else [(0, 2), (2, 4), None]
                lo, hi = pieces[half]

                def emit():
                    ms = slice(4 * c + lo, 4 * c + hi)
                    nc.sync.dma_start(ypart_r[:, ms, :], ypb[:, lo:hi, :])

                return emit

            def rs():
                def emit():
                    nc.gpsimd.collective_compute(
                        "ReduceScatter",
                        mybir.AluOpType.add,
                        replica_groups=[list(range(N_CORES))],
                        ins=[ypart_dram[c * CH : (c + 1) * CH, :]],
                        outs=[y_rs[c * HD : (c + 1) * HD, :]],
                    )

                return emit

            items = []
            for i, mt in enumerate(range(4 * c, 4 * c + 4)):
                for dc in range(NCH):
                    items.extend(group(i, mt, dc))
                if i == 1:
                    items.append(dma_half(0))
                if tail and i == 2:
                    items.append(dma_half(1))
            items.append(dma_half(2 if tail else 1))
            items.append(rs())
            return items

        for c in range(NCH):
            proj_rope_chunk(c, skip_proj=(c == 1))
            for f in vt_fillers(c):
                f()
            filler = []
            if c > 0:
                flush_norm(len(pending_norm))
                filler = ypart_fillers(c - 1)
                for _ in range(6 if c < 3 else 2):
                    filler.pop(0)()
            for h in range(NH_LOC):
                if c == 0:
                    # chunk-1 projection as filler, but only once its xT
                    # chunk has surely landed (a waiting filler blocks the
                    # in-order PE queue)
                    attn_head(h, c, proj1 if h >= 2 else [])
                else:
                    attn_head(h, c, filler)
                if h >= 1:
                    flush_norm(1)
            if c == 0:
                for f in proj1:
                    f()
                del proj1[:]
            for f in filler:
                f()
            del filler[:]
        flush_norm(len(pending_norm))
        for f in ypart_fillers(NCH - 1, tail=True):
            f()
        # per-chunk output copies: the first three wait on already-finished
        # collectives; only the last (64 rows) waits on the final RS
        for c in range(NCH):
            nc.sync.dma_start(
                y_d[c * HD : (c + 1) * HD, :], y_rs[c * HD : (c + 1) * HD, :]
            )


def _prep_shards(x, Wq, lora_A, lora_B, Wk, Wv, Wo):
    bf16 = ml_dtypes.bfloat16
    xT = np.ascontiguousarray(x[0].T).astype(bf16)

    theta = 1.0 / (10000.0 ** (np.arange(0, HD, 2, dtype=np.float32) / HD))
    pos = np.arange(T, dtype=np.float32)
    ang = pos[:, None] * theta[None, :]
    ang = np.concatenate([ang, ang], axis=-1)          # [T, HD]
    cosT = np.cos(ang).T                               # [HD, T]
    sinT = np.sin(ang).T
    sign = np.where(np.arange(HD) < HD // 2, -1.0, 1.0).astype(np.float32)
    sinTs = sinT * sign[:, None]
    cos2 = np.concatenate([cosT, cosT], 0)             # [128, T]
    sin2 = np.concatenate([sinTs, sinTs], 0)

    # RoPE half-rotation permutation (per 64-row head block), symmetric
    perm = np.concatenate(
        [np.arange(32, 64), np.arange(0, 32), np.arange(96, 128), np.arange(64, 96)]
    )
    R = np.eye(P, dtype=np.float32)[perm]

    # additive causal bias for the leading square of each diagonal block:
    # masked iff q_rel < k_rel i.e. f < p
    f_idx = np.arange(P)[None, :]
    p_idx = np.arange(P)[:, None]
    mb = np.where(f_idx < p_idx, NEG, 0.0).astype(np.float32)

    extras = np.ascontiguousarray(
        np.concatenate([R, mb, cos2, sin2], axis=1)
    ).astype(bf16)

    Wq_eff = Wq + lora_B.astype(np.float64) @ lora_A.astype(np.float64)
    Wq_eff = Wq_eff.astype(np.float32)

    in_maps = []
    for i in range(N_CORES):
        wq_i = Wq_eff[QW * i : QW * (i + 1), :]        # [256, D]
        wk_i = Wk[HD * i : HD * (i + 1), :]            # [64, D]
        wv_i = Wv[HD * i : HD * (i + 1), :]
        w_all = np.ascontiguousarray(
            np.concatenate([wq_i, wk_i, wv_i], 0).T
        ).astype(bf16)                                 # [D, 384]
        # Wo columns for this core's heads, transposed: [256, D]
        woT = np.ascontiguousarray(Wo[:, QW * i : QW * (i + 1)].T).astype(bf16)
        in_maps.append({
            "xT": xT,
            "w_all": w_all,
            "woT": woT,
            "extras": extras,
        })
    return in_maps


def run(inputs, trace=False, **kw):
    nc = build_bass()
    in_maps = _prep_shards(**inputs)
    res = run_bass_kernel_spmd(
        nc, in_maps, core_ids=list(range(N_CORES)), trace=trace, **kw
    )
    # core i, chunk c holds final y rows 512c + 64i .. +64
    y = np.zeros((T, D), dtype=np.float32)
    for i in range(N_CORES):
        ri = np.asarray(res.results[i]["y"]).astype(np.float32)
        for c in range(NCH):
            y[CH * c + HD * i : CH * c + HD * (i + 1)] = ri[HD * c : HD * (c + 1)]
    return y[None], res


def kernel(**inputs):
    y, _ = run(inputs)
    return y
